# revision 1
# baseline (speedup 1.0000x reference)
"""MDCA loss kernel for Trainium2 (8 NeuronCores, SPMD data-parallel).

Problem: 4 CAMs [128, 1000, 14, 14] f32 + target [128] i64 ->
4 scalar losses: mean_c |mean_{b,h,w} cam[b,c,h,w] - bincount(target)[c]/B|.

Strategy (memory-bound, ~401 MB total input):
  - Shard batch across 8 cores: 16 rows/core, ~50 MB/core.
  - Per core, per cam: view the [16, 196000] shard as [125p, 16b, 1568]
    where partition p holds classes 8p..8p+7 (1568 = 8 classes * 196 hw,
    contiguous in DRAM -> 6.3 KB DMA runs). CHUNK_B batch rows per DMA
    load; DVE reduce_sum each tile to per-(class, batch-row-group)
    partials, then a tiny second reduce over batch -> per-class sums.
  - One [125, 32] f32 output DMA per core; host sums the 8 core partials,
    adds bincount(target), and computes the 4 scalar losses.

Raw Bass Block (not Tile): HWDGE DMA instructions only support one inline
sync-wait, so semaphores are placed by hand — one completion sem per SBUF
slot (concurrent DMAs always target distinct slots), WAR on slot reuse
guarded transitively through the DVE sem.
"""

import numpy as np

B, C, H, W = 128, 1000, 14, 14
HWSZ = H * W                 # 196
N_CORES = 8
B_SH = B // N_CORES          # 16 batch rows per core
P = 125                      # partitions used; class c -> (p=c//8, cc=c%8)
CC = 8                       # classes per partition
RUN = CC * HWSZ              # 1568 contiguous f32 per (p, b)
F = C * HWSZ                 # 196000 elements per batch row
N_CAMS = 4

CHUNK_B = 1                  # batch rows per load tile
N_BUFS = 12                  # SBUF slots (CHUNK_B*6272 B/partition each)
DUAL_RING = False            # issue loads alternately from sync and scalar HWDGE

_CACHE = {}


def _build_nc(chunk_b=None, n_bufs=None, n_iters=1, dual_ring=None):
    from contextlib import ExitStack

    import concourse.bass as bass
    import concourse.mybir as mybir

    cb = CHUNK_B if chunk_b is None else chunk_b
    nb = N_BUFS if n_bufs is None else n_bufs
    dual = DUAL_RING if dual_ring is None else dual_ring
    n_chunks = B_SH // cb            # loads per cam
    n_loads = N_CAMS * n_chunks      # loads per iteration
    dve_per_iter = n_loads + N_CAMS  # stage1 + stage2 ops per iteration

    def dve_after_s1(k):
        # dve_sem value right after stage1-reduce #k retires (DVE order per
        # cam: n_chunks * s1 then one s2)
        return k + k // n_chunks + 1

    f32 = mybir.dt.float32
    nc = bass.Bass()
    cams = [
        nc.dram_tensor(f"cam_{i}", [B_SH, F], f32, kind="ExternalInput")
        for i in range(N_CAMS)
    ]
    out = nc.dram_tensor("sums", [P, N_CAMS * CC], f32, kind="ExternalOutput")

    with ExitStack() as ctx:
        bufs = [
            ctx.enter_context(nc.sbuf_tensor(f"t{s}", [P, cb, RUN], f32))
            for s in range(nb)
        ]
        stages = [
            ctx.enter_context(nc.sbuf_tensor(f"stage{i}", [P, n_chunks, cb, CC], f32))
            for i in range(N_CAMS)
        ]
        out_sums = ctx.enter_context(nc.sbuf_tensor("osum", [P, N_CAMS * CC], f32))
        # one completion sem per buffer slot: concurrent loads target distinct
        # slots, so "slot_sem >= 16*k" unambiguously means "k-th load into this
        # slot is fully complete" (each DMA is 16 sub-completions)
        slot_sems = [
            ctx.enter_context(nc.semaphore(f"slot_sem{s}")) for s in range(nb)
        ]
        out_sem = ctx.enter_context(nc.semaphore("out_sem"))
        dve_sem = ctx.enter_context(nc.semaphore("dve_sem"))
        block = ctx.enter_context(nc.Block())

        def loader(eng, g, parity):
            # emit this engine's share of iteration g's loads (all, or
            # odd/even when dual-ring); slot-reuse WAR is guarded via
            # dve_sem transitively
            for n in range(n_loads):
                if parity is not None and n % 2 != parity:
                    continue
                i, c = divmod(n, n_chunks)
                gn = g * n_loads + n
                if gn >= nb:
                    # slot's previous tile fully consumed by its stage1
                    # reduce (which also implies that old DMA completed)
                    pk = gn - nb
                    eng.wait_ge(
                        dve_sem,
                        (pk // n_loads) * dve_per_iter
                        + dve_after_s1(pk % n_loads),
                    )
                src = cams[i][c * cb:(c + 1) * cb, :].rearrange(
                    "b (p x) -> p b x", p=P, x=RUN
                )
                eng.dma_start(bufs[gn % nb][:], src).then_inc(
                    slot_sems[gn % nb], 16
                )

        @block.sync
        def _(sync):
            for g in range(n_iters):
                loader(sync, g, 0 if dual else None)
                sync.wait_ge(dve_sem, (g + 1) * dve_per_iter)
                sync.dma_start(out[:, :], out_sums[:]).then_inc(out_sem, 16)
            sync.wait_ge(out_sem, 16 * n_iters)

        if dual:

            @block.scalar
            def _(scalar):
                for g in range(n_iters):
                    loader(scalar, g, 1)

        @block.vector
        def _(vector):
            for g in range(n_iters):
                dve_base = g * dve_per_iter
                for i in range(N_CAMS):
                    for c in range(n_chunks):
                        n = i * n_chunks + c
                        gn = g * n_loads + n
                        if g > 0 and c == 0:
                            # WAR: stages[i] reread by prev iter's stage2
                            vector.wait_ge(
                                dve_sem,
                                (g - 1) * dve_per_iter
                                + (i + 1) * (n_chunks + 1),
                            )
                        vector.wait_ge(
                            slot_sems[gn % nb], 16 * (gn // nb + 1)
                        )
                        nc.vector.reduce_sum(
                            out=stages[i][:, c],
                            in_=bufs[gn % nb][:].rearrange(
                                "p b (cc xx) -> p b cc xx", cc=CC
                            ),
                            axis=mybir.AxisListType.X,
                        ).then_inc(dve_sem, 1)
                    # reduce the 16 batch partials per class:
                    # [P, cc, (chunks b)] -> [P, cc]; same-engine wait makes
                    # sure the stage1 writes retired before this read
                    vector.wait_ge(dve_sem, dve_base + (i + 1) * n_chunks + i)
                    # WAR vs previous iteration's out DMA
                    if g > 0 and i == 0:
                        vector.wait_ge(out_sem, 16 * g)
                    nc.vector.reduce_sum(
                        out=out_sums[:, i * CC:(i + 1) * CC],
                        in_=stages[i][:].rearrange("p h b cc -> p cc (h b)"),
                        axis=mybir.AxisListType.X,
                    ).then_inc(dve_sem, 1)

    return nc


def _get_nc():
    if "nc" not in _CACHE:
        _CACHE["nc"] = _build_nc()
    return _CACHE["nc"]


def _run_on_device(in_maps, nc=None, **kwargs):
    from concourse.bass_utils import run_bass_kernel_spmd

    return run_bass_kernel_spmd(
        nc if nc is not None else _get_nc(),
        in_maps,
        core_ids=list(range(N_CORES)),
        **kwargs,
    )


def _make_in_maps(cams):
    in_maps = []
    for k in range(N_CORES):
        m = {}
        for i, cam in enumerate(cams):
            m[f"cam_{i}"] = np.ascontiguousarray(
                np.asarray(cam)[k * B_SH:(k + 1) * B_SH].reshape(B_SH, F),
                dtype=np.float32,
            )
        in_maps.append(m)
    return in_maps


def kernel(cam_0, cam_1, cam_2, cam_3, target, _bench_results=None, **_kw):
    in_maps = _make_in_maps((cam_0, cam_1, cam_2, cam_3))
    res = _run_on_device(in_maps)
    if _bench_results is not None:
        _bench_results.append(res)

    # host combine: [125, 32] per core -> per-class totals -> scalar losses
    counts = np.bincount(np.asarray(target).astype(np.int64), minlength=C)
    avg_count = counts.astype(np.float64) / B
    total = np.zeros((P, N_CAMS * CC), dtype=np.float64)
    for r in res.results:
        total += r["sums"].astype(np.float64)

    losses = []
    for i in range(N_CAMS):
        per_class = total[:, i * CC:(i + 1) * CC].reshape(C)  # index = 8p+cc = c
        avg_conf = per_class / (B * HWSZ)
        losses.append(np.float32(np.abs(avg_conf - avg_count).mean()))
    return tuple(np.asarray(l, dtype=np.float32) for l in losses)



# revision 12
# speedup vs baseline: 2.0656x; 2.0656x over previous
"""MDCA loss kernel for Trainium2 (8 NeuronCores, SPMD data-parallel).

Problem: 4 CAMs [128, 1000, 14, 14] f32 + target [128] i64 ->
4 scalar losses: mean_c |mean_{b,h,w} cam[b,c,h,w] - bincount(target)[c]/B|.

Strategy (memory-bound, ~401 MB total input):
  - Shard batch across 8 cores: 16 rows/core, ~50 MB/core.
  - Per core, per cam: view the [16, 196000] shard as [125p, 16b, 1568]
    where partition p holds classes 8p..8p+7 (1568 = 8 classes * 196 hw,
    contiguous in DRAM -> 6.3 KB DMA runs). One batch row per DMA load;
    DVE reduce_sum each tile [125, 8, 196] -> [125, 8] per-class partials,
    then a tiny second reduce over the 16 batch rows -> per-class sums.
  - Loads are striped across TWO DMA queues -- the SP (sync) HWDGE ring
    (f32) and the gpsimd SWDGE ring, which CASTS its rows to bf16 in the
    DMA so DVE reduces them in 16-bit 2x mode. Together the rings sustain
    ~850 GB/s/core (one ring alone: ~400); the bf16 tiles halve DVE cost,
    which is otherwise the critical path (f32 full-touch ~94 us/iter).
    The stripe gives the bf16 ring 9/16 of rows to balance DVE against
    both rings. bf16 quantization of ~56% of the inputs perturbs the
    loss by ~6e-5 relative -- far inside the 2e-2 gate. The Activation
    HWDGE ring is ~3.7x slower for bulk loads and carries only the tiny
    output stores, so no load ring ever stalls on DVE.
  - Stage1 uses a 3-D access pattern ([p, cc, xx], contiguous stage
    writes); the 4-D batched form costs ~2x on hardware.
  - One [125, 8] f32 output DMA per cam per core; host sums the 8 core
    partials, adds bincount(target), and computes the 4 scalar losses.

Raw Bass Block (not Tile): HWDGE DMA instructions only support one inline
sync-wait, so semaphores are placed by hand -- one completion sem per SBUF
slot (concurrent DMAs always target distinct slots), WAR on slot reuse
guarded transitively through the DVE sem.
"""

import numpy as np

B, C, H, W = 128, 1000, 14, 14
HWSZ = H * W                 # 196
N_CORES = 8
B_SH = B // N_CORES          # 16 batch rows per core
P = 125                      # partitions used; class c -> (p=c//8, cc=c%8)
CC = 8                       # classes per partition
RUN = CC * HWSZ              # 1568 contiguous f32 per (p, b)
F = C * HWSZ                 # 196000 elements per batch row
N_CAMS = 4

N_BUFS = 16                  # SBUF slots (6272 B/partition each)
LOAD_RINGS = ("sync", "gpsimd")
# ring assignment per (load index % 16): 0 -> sync (f32), 1 -> gpsimd
# (casts to bf16). 7:9 split — the bf16 ring takes slightly more than half
# because its tiles cost DVE half as much, balancing DVE against both DMA
# rings (measured: sync ~400 GB/s, gpsimd ~430 GB/s, DVE ~1 f32 or
# 2 bf16 elems/lane/cycle).
STRIPE = (0, 1, 0, 1, 1, 0, 1, 0, 1, 0, 1, 1, 0, 1, 0, 1)

_CACHE = {}


def _build_nc(n_bufs=None, n_iters=1, load_rings=None, cast_ring=True):
    from contextlib import ExitStack

    import concourse.bass as bass
    import concourse.mybir as mybir

    nb = N_BUFS if n_bufs is None else n_bufs
    rings = LOAD_RINGS if load_rings is None else load_rings
    # cast_ring: the gpsimd (SWDGE) ring casts its rows to bf16 in the DMA;
    # DVE reduces those tiles in 16-bit mode (2x) into f32 stages. Ring
    # assignment follows STRIPE (period 16); nb must be 16 so that a slot
    # is always refilled by the same ring (same dtype).
    assert not cast_ring or (
        len(rings) == 2 and rings[1] == "gpsimd" and nb == 16
    )
    n_chunks = B_SH                  # loads per cam (one batch row each)
    n_loads = N_CAMS * n_chunks      # loads per iteration
    dve_per_iter = n_loads + N_CAMS  # stage1 + stage2 ops per iteration

    def dve_after_s1(k):
        # dve_sem value right after stage1-reduce #k retires (DVE order per
        # cam: n_chunks * s1 then one s2)
        return k + k // n_chunks + 1

    def dve_after_s2(i):
        # dve_sem delta within an iteration once cam i's stage2 retired
        return (i + 1) * (n_chunks + 1)

    f32 = mybir.dt.float32
    bf16 = mybir.dt.bfloat16
    nc = bass.Bass()
    cams = [
        nc.dram_tensor(f"cam_{i}", [B_SH, F], f32, kind="ExternalInput")
        for i in range(N_CAMS)
    ]
    out = nc.dram_tensor("sums", [P, N_CAMS * CC], f32, kind="ExternalOutput")

    def ring_of(n):
        if cast_ring:
            return STRIPE[n % 16]
        return n % len(rings)

    def slot_dtype(s):
        # slot s is always filled by ring STRIPE[s % 16] (n_loads % 16 == 0
        # and nb == 16, so load n -> slot n % 16 preserves the stripe)
        return bf16 if cast_ring and STRIPE[s % 16] == 1 else f32

    with ExitStack() as ctx:
        bufs = [
            ctx.enter_context(nc.sbuf_tensor(f"t{s}", [P, RUN], slot_dtype(s)))
            for s in range(nb)
        ]
        stages = [
            ctx.enter_context(nc.sbuf_tensor(f"stage{i}", [P, n_chunks, CC], f32))
            for i in range(N_CAMS)
        ]
        out_sums = ctx.enter_context(nc.sbuf_tensor("osum", [P, N_CAMS * CC], f32))
        # one completion sem per buffer slot: concurrent loads target distinct
        # slots, so "slot_sem >= 16*k" unambiguously means "k-th load into this
        # slot is fully complete" (each DMA is 16 sub-completions)
        slot_sems = [
            ctx.enter_context(nc.semaphore(f"slot_sem{s}")) for s in range(nb)
        ]
        out_sem = ctx.enter_context(nc.semaphore("out_sem"))
        dve_sem = ctx.enter_context(nc.semaphore("dve_sem"))
        block = ctx.enter_context(nc.Block())

        def loader(eng, g, parity, nrings):
            # emit this engine's share of iteration g's loads (striped);
            # slot-reuse WAR is guarded via dve_sem transitively
            for n in range(n_loads):
                if ring_of(n) != parity:
                    continue
                i, c = divmod(n, n_chunks)
                gn = g * n_loads + n
                if gn >= nb:
                    # slot's previous tile fully consumed by its stage1
                    # reduce (which also implies that old DMA completed)
                    pk = gn - nb
                    eng.wait_ge(
                        dve_sem,
                        (pk // n_loads) * dve_per_iter
                        + dve_after_s1(pk % n_loads),
                    )
                src = cams[i][c:c + 1, :].rearrange(
                    "b (p x) -> p (b x)", p=P, x=RUN
                )
                eng.dma_start(bufs[gn % nb][:], src).then_inc(
                    slot_sems[gn % nb], 16
                )

        for r, ring in enumerate(rings):

            def ring_body(eng, r=r):
                for g in range(n_iters):
                    loader(eng, g, r, len(rings))

            getattr(block, ring)(ring_body)

        @block.scalar
        def _(scalar):
            # per-cam output stores on the otherwise-idle Act ring: wait for
            # cam i's stage2, then DMA its 8 columns out
            for g in range(n_iters):
                for i in range(N_CAMS):
                    scalar.wait_ge(
                        dve_sem, g * dve_per_iter + dve_after_s2(i)
                    )
                    scalar.dma_start(
                        out[:, i * CC:(i + 1) * CC],
                        out_sums[:, i * CC:(i + 1) * CC],
                    ).then_inc(out_sem, 16)
            scalar.wait_ge(out_sem, 16 * N_CAMS * n_iters)

        @block.vector
        def _(vector):
            for g in range(n_iters):
                dve_base = g * dve_per_iter
                for i in range(N_CAMS):
                    for c in range(n_chunks):
                        n = i * n_chunks + c
                        gn = g * n_loads + n
                        if g > 0 and c == 0:
                            # WAR: stages[i] reread by prev iter's stage2
                            vector.wait_ge(
                                dve_sem,
                                (g - 1) * dve_per_iter + dve_after_s2(i),
                            )
                        vector.wait_ge(
                            slot_sems[gn % nb], 16 * (gn // nb + 1)
                        )
                        nc.vector.reduce_sum(
                            out=stages[i][:, c],
                            in_=bufs[gn % nb][:].rearrange(
                                "p (cc xx) -> p cc xx", cc=CC
                            ),
                            axis=mybir.AxisListType.X,
                        ).then_inc(dve_sem, 1)
                    # reduce the 16 batch partials per class:
                    # [P, cc, h] -> [P, cc]; same-engine wait makes sure the
                    # stage1 writes retired before this read
                    vector.wait_ge(dve_sem, dve_base + (i + 1) * n_chunks + i)
                    # WAR vs previous iteration's out DMA of this cam's cols
                    if g > 0:
                        vector.wait_ge(out_sem, 16 * (N_CAMS * (g - 1) + i + 1))
                    nc.vector.reduce_sum(
                        out=out_sums[:, i * CC:(i + 1) * CC],
                        in_=stages[i][:].rearrange("p h cc -> p cc h"),
                        axis=mybir.AxisListType.X,
                    ).then_inc(dve_sem, 1)

    return nc


def _get_nc():
    if "nc" not in _CACHE:
        _CACHE["nc"] = _build_nc()
    return _CACHE["nc"]


def _run_on_device(in_maps, nc=None, **kwargs):
    from concourse.bass_utils import run_bass_kernel_spmd

    return run_bass_kernel_spmd(
        nc if nc is not None else _get_nc(),
        in_maps,
        core_ids=list(range(N_CORES)),
        **kwargs,
    )


def _make_in_maps(cams):
    in_maps = []
    for k in range(N_CORES):
        m = {}
        for i, cam in enumerate(cams):
            m[f"cam_{i}"] = np.ascontiguousarray(
                np.asarray(cam)[k * B_SH:(k + 1) * B_SH].reshape(B_SH, F),
                dtype=np.float32,
            )
        in_maps.append(m)
    return in_maps


def kernel(cam_0, cam_1, cam_2, cam_3, target, _bench_results=None, **_kw):
    in_maps = _make_in_maps((cam_0, cam_1, cam_2, cam_3))
    res = _run_on_device(in_maps)
    if _bench_results is not None:
        _bench_results.append(res)

    # host combine: [125, 32] per core -> per-class totals -> scalar losses
    counts = np.bincount(np.asarray(target).astype(np.int64), minlength=C)
    avg_count = counts.astype(np.float64) / B
    total = np.zeros((P, N_CAMS * CC), dtype=np.float64)
    for r in res.results:
        total += r["sums"].astype(np.float64)

    losses = []
    for i in range(N_CAMS):
        per_class = total[:, i * CC:(i + 1) * CC].reshape(C)  # index = 8p+cc = c
        avg_conf = per_class / (B * HWSZ)
        losses.append(np.float32(np.abs(avg_conf - avg_count).mean()))
    return tuple(np.asarray(l, dtype=np.float32) for l in losses)
